# revision 5
# baseline (speedup 1.0000x reference)
"""Trainium2 Bass kernel for nn_ConvNet01 (conv5x5 -> Hermite poly -> maxpool2
-> FC), data-parallel over 8 NeuronCores.

Self-contained: hardcodes shapes/layout; builds device-format arrays on host,
compiles + runs a Bass/Tile kernel via run_bass_kernel_spmd, gathers output.

Per-core layout (BC=512 images):
  conv as one matmul per (panel a, block t): window (gh,gw)=(2,3),
  K = 126 im2col rows (c,dh,dw) + ones row = 127, M = 96 (gh'*48+gw'*16+oc),
  N = batch. oh = 2a+gh', ow = 3t+gw' (ow 28,29 junk, never pooled).
  Hermite activation p(y) = kappa*body + shift with
  body = ((u*A+B)*u + C)*y + u*u, u = (s*y+t)^2  [ACT Square + 1 custom DVE op]
  maxpool: gh'-pairs via SBUF->SBUF DMA with CCE max/min (partition crossing),
  ow-pairs via strided DVE tensor_tensor max/min. FC absorbs kappa/shift.
"""
import numpy as np

import concourse.bacc as bacc
import concourse.bass as bass
import concourse.mybir as mybir
import concourse.dve_ops as dve_ops
from concourse.dve_spec import Spec, Src0, Src1, C0, C1, C2, lower as dve_lower
from concourse.dve_spec import _has_src1
from concourse.dve_uop import DveOpSpec
from concourse.tile import TileContext
from concourse.bass_utils import run_bass_kernel_spmd
from concourse._compat import get_trn_type

B, C, H, W = 4096, 3, 32, 32
NCORES = 8
BC = B // NCORES
NPANEL, NT = 14, 10
KP, M = 127, 96
ACHUNK, NCHUNKS = 4, 4
F16 = mybir.dt.float16
F32 = mybir.dt.float32

# ---------------------------------------------------------------- host math

def _herm2mono(coef):
    n = len(coef)
    Hm = np.zeros((n, n))
    Hm[0, 0] = 1.0
    if n > 1:
        Hm[1, 1] = 2.0
    for k in range(2, n):
        Hm[k, 1:] += 2.0 * Hm[k - 1, :-1]
        Hm[k, :] -= 2.0 * (k - 1) * Hm[k - 2, :]
    return (np.asarray(coef, dtype=np.float64)[:, None] * Hm).sum(axis=0)


def _solve_fused(c):
    c = np.asarray(c, dtype=np.float64)
    c0, c1, c2, c3, c4, c5 = c
    if abs(c5) < 1e-30:
        return None

    def try_kappa(kappa):
        def resid(s):
            A = c5 / (kappa * s**4)
            t = (c4 / kappa - s**4) / (4 * A * s**3)
            P2 = np.array([t * t, 2 * s * t, s * s])
            P4 = np.convolve(P2, P2)
            Bc = (c3 / kappa - A * P4[2] - P4[3]) / P2[2]
            r = A * P4[1] + Bc * P2[1] + P4[2] - c2 / kappa
            return r, A, Bc, t, P2, P4

        ss = np.concatenate([-np.geomspace(3.0, 1e-3, 400),
                             np.geomspace(1e-3, 3.0, 400)])
        vals = []
        for s in ss:
            try:
                vals.append(resid(s)[0])
            except (ZeroDivisionError, FloatingPointError):
                vals.append(np.nan)
        vals = np.array(vals)
        best = None
        for i in range(len(ss) - 1):
            a, b = vals[i], vals[i + 1]
            if np.isnan(a) or np.isnan(b) or a * b > 0:
                continue
            lo, hi, flo = ss[i], ss[i + 1], a
            for _ in range(200):
                mid = 0.5 * (lo + hi)
                fm = resid(mid)[0]
                if flo * fm <= 0:
                    hi = mid
                else:
                    lo, flo = mid, fm
            s = 0.5 * (lo + hi)
            r, A, Bc, t, P2, P4 = resid(s)
            Cc = c1 / kappa - A * P4[0] - Bc * P2[0] - P4[1]
            shift = c0 - kappa * P4[0]
            if max(abs(A), abs(Bc), abs(Cc)) < 1e6 and abs(r) < 1e-8:
                cand = dict(s=s, t=t, A=A, B=Bc, C=Cc, kappa=kappa, shift=shift)
                if best is None or abs(A) < abs(best["A"]):
                    best = cand
        return best

    best = None
    for kappa in [c4, c5, -c5, 1.0, -1.0, c5 * 10, c5 * 0.1, -c5 * 10,
                  -c5 * 0.1, c4 * 10, c4 * 0.1]:
        if abs(kappa) < 1e-12:
            continue
        sol = try_kappa(kappa)
        if sol is not None and (best is None or abs(sol["A"]) < abs(best["A"])):
            best = sol
    return best


def build_host_arrays(x, coef, conv_w, conv_b, fc_w, fc_b):
    x = np.asarray(x, np.float32)
    conv_w = np.asarray(conv_w, np.float32)
    conv_b = np.asarray(conv_b, np.float32)
    fc_w = np.asarray(fc_w, np.float32)
    fc_b = np.asarray(fc_b, np.float32)

    sol = _solve_fused(_herm2mono(np.asarray(coef, np.float64)))
    assert sol is not None, "fused poly solve failed"
    kappa, shift = sol["kappa"], sol["shift"]

    xpad = np.zeros((B, C, H, W + 2), np.float32)
    xpad[..., :W] = x
    cs = np.arange(126) // 42
    dhs = (np.arange(126) % 42) // 7
    dws = np.arange(126) % 7
    hidx = 2 * np.arange(NPANEL)[:, None] + dhs[None, :]
    widx = 3 * np.arange(NT)[None, :] + dws[:, None]
    g = xpad[np.arange(B)[:, None, None, None], cs[None, None, :, None],
             hidx[None, :, :, None], widx[None, None, :, :]]
    X4 = np.ones((B, NPANEL, KP, NT), np.float32)
    X4[:, :, :126, :] = g
    X4 = X4.reshape(NCORES, BC, NPANEL, KP, NT).transpose(0, 2, 3, 4, 1)
    X4 = np.ascontiguousarray(X4, dtype=np.float16)  # [core, a, p, s, b]

    lhsT = np.zeros((KP, M), np.float32)
    for p in range(126):
        c, r = divmod(p, 42)
        dh, dw = divmod(r, 7)
        for ghp in range(2):
            kh = dh - ghp
            if not (0 <= kh < 5):
                continue
            for gwp in range(3):
                kw = dw - gwp
                if not (0 <= kw < 5):
                    continue
                m = ghp * 48 + gwp * 16 + np.arange(16)
                lhsT[p, m] = conv_w[:, c, kh, kw]
    lhsT[126, :] = np.tile(conv_b, 6)
    lhsT = lhsT.astype(np.float16)

    # fcw packed: [64, 56*10] fp16 (partition-major for SBUF tile)
    fcw = np.zeros((64, NCHUNKS * 14, 10), np.float32)
    for chunk in range(NCHUNKS):
        for owp in range(14):
            q = chunk * 14 + owp
            for a_sub in range(ACHUNK):
                a = chunk * ACHUNK + a_sub
                if a >= NPANEL:
                    continue
                for oc in range(16):
                    f = oc * 196 + a * 14 + owp
                    fcw[a_sub * 16 + oc, q, :] = kappa * fc_w[:, f]
    fcw = fcw.reshape(64, NCHUNKS * 14 * 10).astype(np.float16)
    b_eff = (fc_b.astype(np.float64)
             + shift * fc_w.astype(np.float64).sum(axis=1)).astype(np.float32)

    return dict(X4=X4, lhsT=lhsT, fcw=fcw, b_eff=b_eff.reshape(10, 1),
                sol=sol, use_min=kappa < 0)


# ---------------------------------------------------------- custom DVE op

_POLY_OP = None


def _get_poly_op():
    global _POLY_OP
    if _POLY_OP is not None:
        return _POLY_OP
    name = "POLY54_ANT"
    for op in dve_ops.OPS:
        if op.name == name:
            _POLY_OP = op
            return op
    body = ((Src0 * C0 + C1) * Src0 + C2) * Src1 + Src0 * Src0
    spec = Spec(
        body=body,
        reference=lambda in0, in1, s0, s1, imm2: (
            ((in0.astype(np.float32) * s0 + s1) * in0 + imm2) * in1
            + in0.astype(np.float32) * in0
        ),
    )
    row = max(dve_ops._SUB_OPCODE_FOR_NAME.values()) + 1
    assert row < 0x20
    shas = {}
    for ver in ("v3", "v4"):
        try:
            uops = dve_lower(spec, ver=ver)
            s = DveOpSpec(name=name, opcode=row, uops=uops,
                          rd1_en=_has_src1(spec))
            shas[ver] = s.sha(ver)
        except Exception:
            pass
    assert "v3" in shas, "POLY op failed to lower for v3"
    op = dve_ops.DveOp(name, spec, subdim=False, uops_sha=shas)
    dve_ops.OPS.append(op)
    dve_ops._SUB_OPCODE_FOR_NAME[name] = row
    dve_ops.CUSTOM_DVE_SPECS[name] = spec
    _POLY_OP = op
    return op


# ------------------------------------------------------------- bass build

def build_nc(sol, use_min, repeat=1):
    poly_op = _get_poly_op()
    ss, tt = float(sol["s"]), float(sol["t"])
    Ac, Bc, Cc = float(sol["A"]), float(sol["B"]), float(sol["C"])
    pool_alu = mybir.AluOpType.min if use_min else mybir.AluOpType.max

    nc = bacc.Bacc(get_trn_type() or "TRN2", target_bir_lowering=False,
                   debug=False)
    X4d = nc.dram_tensor("x4", [NPANEL, KP, NT * BC], F16,
                         kind="ExternalInput")
    lhsTd = nc.dram_tensor("lhst", [KP, M], F16, kind="ExternalInput")
    fcwd = nc.dram_tensor("fcw", [64, NCHUNKS * 14 * 10], F16,
                          kind="ExternalInput")
    beffd = nc.dram_tensor("beff", [10, 1], F32, kind="ExternalInput")
    tbias = nc.dram_tensor("tbias", [128, 1], F32, kind="ExternalInput")
    outd = nc.dram_tensor("out", [10, BC], F32, kind="ExternalOutput")

    with TileContext(nc) as tc:
        with tc.tile_pool(name="const", bufs=1) as cpool, \
             tc.tile_pool(name="panel", bufs=2) as panel_pool, \
             tc.tile_pool(name="upool", bufs=3) as upool, \
             tc.tile_pool(name="pchunk", bufs=3) as ppool, \
             tc.tile_pool(name="hm0", bufs=1) as hm0pool, \
             tc.tile_pool(name="hm1", bufs=1) as hm1pool, \
             tc.tile_pool(name="wc", bufs=1) as wcpool, \
             tc.tile_pool(name="osb", bufs=1) as opool, \
             tc.tile_pool(name="psum", bufs=3, space="PSUM") as pspool, \
             tc.tile_pool(name="fcps", bufs=1, space="PSUM") as fcpspool:

            lhsT_sb = cpool.tile([KP, M], F16)
            nc.sync.dma_start(out=lhsT_sb, in_=lhsTd[:, :])
            fcw_sb = cpool.tile([64, NCHUNKS * 14 * 10], F16)
            nc.sync.dma_start(out=fcw_sb, in_=fcwd[:, :])
            beff_sb = cpool.tile([10, 1], F32)
            nc.sync.dma_start(out=beff_sb, in_=beffd[:, :])
            tbias_sb = cpool.tile([128, 1], F32)
            nc.sync.dma_start(out=tbias_sb, in_=tbias[:, :])

            def body(_i=None):
                fc_ps = fcpspool.tile([10, BC], F32)
                for chunk in range(NCHUNKS):
                    na = min(ACHUNK, NPANEL - chunk * ACHUNK)
                    np_parts = na * 16
                    hm0 = hm0pool.tile([64, 30 * BC], F16)
                    hm1 = hm1pool.tile([64, 30 * BC], F16)
                    for a_sub in range(na):
                        a = chunk * ACHUNK + a_sub
                        panel = panel_pool.tile([KP, NT * BC], F16)
                        nc.sync.dma_start(out=panel, in_=X4d[a])
                        pchunk = ppool.tile([M, NT * BC], F16)
                        for tp in range(5):
                            ps = pspool.tile([M, 1024], F32)
                            for half in range(2):
                                nc.tensor.matmul(
                                    ps[:, half * 512:(half + 1) * 512],
                                    lhsT_sb[:, :],
                                    panel[:, tp * 1024 + half * 512:
                                          tp * 1024 + (half + 1) * 512],
                                    start=True, stop=True)
                            u = upool.tile([M, 1024], F32)
                            nc.scalar.activation(
                                u, ps, mybir.ActivationFunctionType.Square,
                                bias=tbias_sb[0:M, 0:1], scale=ss)
                            off = tp * 1024
                            nc.vector._custom_dve(
                                poly_op, out=pchunk[:, off:off + 1024],
                                in0=u, in1=ps, s0=Ac, s1=Bc, imm2=Cc)
                        # relayout copies: gh'=0 -> hm0, gh'=1 -> hm1
                        for ghp in range(2):
                            for gwp in range(3):
                                p0 = ghp * 48 + gwp * 16
                                src = pchunk[p0:p0 + 16, :]
                                dsttile = hm0 if ghp == 0 else hm1
                                dst = dsttile[a_sub * 16:(a_sub + 1) * 16, :] \
                                    .rearrange("p (t three b) -> p t three b",
                                               t=NT, three=3, b=BC)[:, :, gwp, :]
                                nc.sync.dma_start(out=dst, in_=src)
                    # h-pool: in-place max into hm0 (fp16 dense, 2x mode)
                    nc.vector.tensor_tensor(
                        out=hm0[0:np_parts, :], in0=hm0[0:np_parts, :],
                        in1=hm1[0:np_parts, :], op=pool_alu)
                    # w-pool over ow pairs (0..27)
                    wc = wcpool.tile([64, 14 * BC], F16)
                    hmv = hm0[0:np_parts, :].rearrange(
                        "p (owp two b) -> p owp two b", owp=15, two=2, b=BC)
                    nc.vector.tensor_tensor(
                        out=wc[0:np_parts, :].rearrange(
                            "p (owp b) -> p owp b", owp=14, b=BC),
                        in0=hmv[:, 0:14, 0, :], in1=hmv[:, 0:14, 1, :],
                        op=pool_alu)
                    for owp in range(14):
                        q = chunk * 14 + owp
                        nc.tensor.matmul(
                            fc_ps,
                            fcw_sb[0:np_parts, q * 10:(q + 1) * 10],
                            wc[0:np_parts, owp * BC:(owp + 1) * BC],
                            start=(q == 0), stop=(q == NCHUNKS * 14 - 1))
                out_sb = opool.tile([10, BC], F32)
                nc.scalar.activation(out_sb, fc_ps,
                                     mybir.ActivationFunctionType.Identity,
                                     bias=beff_sb[:, 0:1], scale=1.0)
                nc.sync.dma_start(out=outd[:, :], in_=out_sb)

            if repeat == 1:
                body()
            else:
                with tc.For_i(0, repeat, 1) as i:
                    body(i)

    nc.compile()
    return nc


# --------------------------------------------------------------- entry

def kernel(x, coef, conv_w, conv_b, fc_w, fc_b):
    ha = build_host_arrays(x, coef, conv_w, conv_b, fc_w, fc_b)
    nc = build_nc(ha["sol"], ha["use_min"], repeat=1)
    in_maps = []
    for core in range(NCORES):
        in_maps.append({
            "x4": ha["X4"][core].reshape(NPANEL, KP, NT * BC),
            "lhst": ha["lhsT"],
            "fcw": ha["fcw"],
            "beff": ha["b_eff"],
            "tbias": np.full((128, 1), np.float32(ha["sol"]["t"]),
                             dtype=np.float32),
        })
    res = run_bass_kernel_spmd(nc, in_maps, core_ids=list(range(NCORES)))
    out = np.concatenate([res.results[c]["out"].T for c in range(NCORES)],
                         axis=0)
    return out.astype(np.float32)


if __name__ == "__main__":
    import reference
    inputs = {k: np.asarray(v) for k, v in reference.setup_inputs().items()}
    expected = np.asarray(reference.reference(**inputs))
    got = kernel(**inputs)
    err = np.abs(got - expected)
    scale = np.abs(expected).max()
    print(f"max abs err {err.max():.4e}  scale {scale:.3g}  "
          f"rel {err.max() / scale:.4e}")
